# revision 30
# baseline (speedup 1.0000x reference)
"""Trainium2 Bass kernel: per-combination linear encoder via PE one-hot
matmuls, v2 (row-tiled S1 + multi-engine stage 2).

z = y * w[idx] + b[idx],  idx = t*1024 + x @ [512..1]  (11 bits, 2048 combos)

Split idx = hi5*64 + lo6 (hi5 = t,x0..x3; lo6 = x4..x9). Per pair of
tiles (2 x 1024 rows; each tile = 2 groups g of 512 columns, rows on
the FREE axis):

  S1  PE   4 concurrent 32x128 row-tiled fp8 matmuls (tile_position
           quadrants 0/32/64/96) compute uu = [u_lo | u_hi] for both
           tiles of the pair at once, where u_lo[64g+l,c] = lo6-l and
           u_hi[64g+2h+e,c] = hi5-h. Features are replicated in 4 SBUF
           partition quadrants (fp8, so DMA bytes match the old bf16
           single-copy layout). The -l / -h stationary entries are
           split over two ones-features so every value is fp8-exact.
  ACT      drains uu psum -> bf16 SBUF (exact: |u| < 64)
  DVE      oh = (u_lo == 0)            bf16 one-hot (4x mode)
  PE       V[64g+2h+e, c] = sum_l T[l,2h+e]*oh  (T = w/b tables, bf16)
  DVE      msk = (u_hi == 0) * V       fused scalar_tensor_tensor,
                                       u_hi from SBUF, V from psum
  PE       sel8[48,512] accumulates 8 tiles per pack via slot-shifted
           one-hot stationaries (slot 2u+g for w, 32+2u+g for b)
  DVE      z[16,512] = y16 * sel8[0:16] + sel8[32:48]; DMA out.

PE work is software-pipelined: at iteration j the PE runs V(j-1),
S1(j), sel(j-2) so no engine blocks on the serial chain, and the
pack-tail fma is deferred one DVE op per iteration so it never stalls
the critical one-hot/STT chain. PSUM budget: uu 2x2 + V 2 + sel 2 =
8 banks. ACT (the uu drain, ~2.2us/pair) is the pacing engine; DVE is
~2.1us/pair; PE ~2.0us/pair.

All row/column packing is host-side layout only; the device does all
arithmetic. w/b are bf16 (0.2% rounding, tolerance is 2e-2).
"""

import numpy as np
import ml_dtypes

import concourse.bacc as bacc
import concourse.mybir as mybir
from concourse.tile import TileContext
from concourse.bass_utils import run_bass_kernel_spmd

M = 8              # NeuronCores
NT = 512           # columns per tile (one PSUM bank)
G = 2              # row-groups per column
TPP = 8            # tiles per pack (sel8 accumulation group)
NPACK = 31         # packs per core
NTILES = NPACK * TPP          # 248
NPAIR = NTILES // 2           # 124
RPT = G * NT                  # rows per tile (1024)
R = NTILES * RPT              # rows per core (253952)
C = 2048
NF = 13            # features per group: t, x0..x9, one, one
F32 = mybir.dt.float32
BF16 = mybir.dt.bfloat16
FP8 = mybir.dt.float8e4
BF = ml_dtypes.bfloat16
F8 = ml_dtypes.float8_e4m3

OH_ON_GPSIMD = False

_CACHE = {}


def _build_program():
    nc = bacc.Bacc("TRN2", target_bir_lowering=False, debug=False, num_devices=M)

    xin = nc.dram_tensor("xin", [128, NPAIR * NT], FP8, kind="ExternalInput")
    yin = nc.dram_tensor("yin", [NPACK * 16, NT], F32, kind="ExternalInput")
    a12 = nc.dram_tensor("a12", [128, 128], FP8, kind="ExternalInput")
    a3 = nc.dram_tensor("a3", [128, 128], BF16, kind="ExternalInput")
    a4 = nc.dram_tensor("a4", [128, TPP * 48], BF16, kind="ExternalInput")
    z = nc.dram_tensor("z", [NPACK * 16, NT], F32, kind="ExternalOutput")

    isq = mybir.AluOpType.is_equal
    mul = mybir.AluOpType.mult
    add = mybir.AluOpType.add

    with TileContext(nc) as tc:
        with (
            tc.tile_pool(name="const", bufs=1) as cpool,
            tc.tile_pool(name="inx", bufs=3) as ipool,
            tc.tile_pool(name="iny", bufs=3) as ypool,
            tc.tile_pool(name="mid", bufs=4) as spool,
            tc.tile_pool(name="ohp", bufs=4) as ohpool,
            tc.tile_pool(name="msk", bufs=6) as mpool,
            tc.tile_pool(name="out", bufs=2) as opool,
            tc.tile_pool(name="psu", bufs=1, space="PSUM") as ppu,
            tc.tile_pool(name="psv", bufs=2, space="PSUM") as ppv,
            tc.tile_pool(name="pss", bufs=2, space="PSUM") as pps,
        ):
            a12_t = cpool.tile([128, 128], FP8)
            nc.sync.dma_start(out=a12_t[:], in_=a12[:, :])
            a3_t = cpool.tile([128, 128], BF16)
            nc.sync.dma_start(out=a3_t[:], in_=a3[:, :])
            a4_t = cpool.tile([128, TPP * 48], BF16)
            nc.sync.dma_start(out=a4_t[:], in_=a4[:, :])

            xts = {}
            yts = {}

            def fetch_pack(k):
                xt = ipool.tile([128, (TPP // 2) * NT], FP8, tag="x", name="xt")
                base = k * (TPP // 2) * NT
                for s in range(4):
                    nc.sync.dma_start(
                        out=xt[:, s * NT:(s + 1) * NT],
                        in_=xin[:, base + s * NT:base + (s + 1) * NT],
                    )
                yt = ypool.tile([16, NT], F32, tag="y", name="yt")
                nc.sync.dma_start(out=yt[:], in_=yin[16 * k:16 * (k + 1), :])
                xts[k] = xt
                yts[k] = yt

            sel8_box = [None]
            tail_q = []

            def issue_tail(s8, yt_, pk_):
                # Deferred: one DVE op per later iteration so the pack
                # tail never stalls the critical one-hot/STT chain.
                zbox = [None]

                def c1():
                    zbox[0] = opool.tile([16, NT], F32, tag="z", name="zt")
                    nc.vector.tensor_tensor(
                        out=zbox[0][:], in0=yt_[:], in1=s8[0:16, :], op=mul
                    )

                def c2():
                    nc.vector.tensor_tensor(
                        out=zbox[0][:], in0=zbox[0][:], in1=s8[32:48, :], op=add
                    )
                    nc.sync.dma_start(
                        out=z[16 * pk_:16 * (pk_ + 1), :], in_=zbox[0][:]
                    )

                tail_q.append(c1)
                tail_q.append(c2)

            def issue_V(st):
                oh_l, uub, u0, yt_, pk_ = st
                V_l = [ppv.tile([128, NT], F32, tag="V", name="V") for _ in range(2)]
                msk_l = [
                    mpool.tile([128, NT], BF16, tag="msk", name="msk") for _ in range(2)
                ]
                for i in range(2):
                    nc.tensor.matmul(
                        out=V_l[i][:], lhsT=a3_t[:], rhs=oh_l[i][:],
                        start=True, stop=True,
                    )
                for i in range(2):
                    nc.vector.scalar_tensor_tensor(
                        out=msk_l[i][:],
                        in0=uub[:, (2 * i + 1) * NT:(2 * i + 2) * NT], scalar=0.0,
                        in1=V_l[i][:], op0=isq, op1=mul,
                    )
                return (msk_l, u0, yt_, pk_)

            def issue_sel(st):
                msk_l, u0, yt_, pk_ = st
                for i in range(2):
                    u = (u0 + i) % TPP
                    if u == 0:
                        sel8_box[0] = pps.tile(
                            [64, NT], F32, tag="sel", name="sel8"
                        )
                    nc.tensor.matmul(
                        out=sel8_box[0][0:48, :],
                        lhsT=a4_t[:, 48 * u:48 * (u + 1)],
                        rhs=msk_l[i][:],
                        start=(u == 0), stop=(u == TPP - 1),
                    )
                if (u0 + 1) % TPP == TPP - 1:
                    issue_tail(sel8_box[0], yt_, pk_)

            pend_v = None
            pend_sel = None

            fetch_pack(0)
            for pk in range(NPACK):
                if pk + 1 < NPACK:
                    fetch_pack(pk + 1)
                xt = xts.pop(pk)
                yt = yts[pk]

                for p in range(TPP // 2):
                    u0 = pk * TPP + 2 * p

                    new_sel = issue_V(pend_v) if pend_v is not None else None

                    # S1: 4 concurrent row-tiled matmuls for this pair into
                    # ONE [128, 2048] psum tile: [loA | hiA | loB | hiB].
                    xq = xt[:, p * NT:(p + 1) * NT]
                    uu = ppu.tile([128, 4 * NT], F32, tag="uu", name="uu")
                    for i in range(2):
                        qb = 64 * i
                        nc.tensor.matmul(
                            out=uu[:, (2 * i) * NT:(2 * i + 1) * NT],
                            lhsT=a12_t[qb:qb + 2 * NF, :],
                            rhs=xq[qb:qb + 2 * NF, :],
                            start=True, stop=True, tile_position=(qb, 0),
                        )
                        nc.tensor.matmul(
                            out=uu[:, (2 * i + 1) * NT:(2 * i + 2) * NT],
                            lhsT=a12_t[qb + 32:qb + 32 + 2 * NF, :],
                            rhs=xq[qb + 32:qb + 32 + 2 * NF, :],
                            start=True, stop=True, tile_position=(qb + 32, 0),
                        )

                    if pend_sel is not None:
                        issue_sel(pend_sel)
                    pend_sel = new_sel

                    # Single ACT drain for the whole pair (FD=2048), then
                    # per-tile one-hot compares on the lo planes.
                    uub = spool.tile([128, 4 * NT], BF16, tag="uub", name="uub")
                    nc.scalar.copy(out=uub[:], in_=uu[:])
                    oh_l = []
                    for i in range(2):
                        oh = ohpool.tile([128, NT], BF16, tag="oh", name="oh")
                        nc.vector.tensor_scalar(
                            out=oh[:], in0=uub[:, (2 * i) * NT:(2 * i + 1) * NT],
                            scalar1=0.0, scalar2=None, op0=isq,
                        )
                        oh_l.append(oh)

                    if tail_q:
                        tail_q.pop(0)()

                    pend_v = (oh_l, uub, u0, yt, pk)

            # drain the software pipeline
            new_sel = issue_V(pend_v)
            issue_sel(pend_sel)
            issue_sel(new_sel)
            while tail_q:
                tail_q.pop(0)()

    nc.compile()
    return nc


def _get_program():
    if "nc" not in _CACHE:
        _CACHE["nc"] = _build_program()
    return _CACHE["nc"]


def _make_consts(w, b):
    f32 = np.float32
    wb_ = np.stack([np.asarray(w, f32), np.asarray(b, f32)], 1).astype(BF)  # [2048, 2]

    # a12: 4 quadrants of fp8 stationaries. Quadrants 0/2: a1 (lo6),
    # quadrants 1/3: a2 (hi5). All entries fp8-exact.
    a12 = np.zeros((128, 128), f32)
    lvals = np.arange(64, dtype=f32)
    hvals = np.repeat(np.arange(32, dtype=f32), 2)
    for q in range(4):
        qb = 32 * q
        for g in range(G):
            cb = 64 * g
            if q % 2 == 0:  # a1: lo6 from x4..x9 (feature rows 5..10)
                for si in range(6):
                    a12[qb + NF * g + 5 + si, cb:cb + 64] = 2.0 ** (5 - si)
                a12[qb + NF * g + 11, cb:cb + 64] = -(lvals - (lvals % 4))
                a12[qb + NF * g + 12, cb:cb + 64] = -(lvals % 4)
            else:  # a2: hi5 from t,x0..x3 (feature rows 0..4)
                for si in range(5):
                    a12[qb + NF * g + si, cb:cb + 64] = 2.0 ** (4 - si)
                a12[qb + NF * g + 11, cb:cb + 64] = -(hvals - (hvals % 4))
                a12[qb + NF * g + 12, cb:cb + 64] = -(hvals % 4)
    a12 = a12.astype(F8)

    a3 = np.zeros((128, 128), BF)
    for g in range(G):
        for h in range(32):
            for e in range(2):
                a3[64 * g:64 * g + 64, 64 * g + 2 * h + e] = wb_[h * 64:(h + 1) * 64, e]
    # sel8 slot for tile u, group g: w at partition 2u+g, b at 32+2u+g
    a4 = np.zeros((128, TPP * 48), BF)
    for u in range(TPP):
        for g in range(G):
            for e in range(2):
                for h in range(32):
                    a4[64 * g + 2 * h + e, 48 * u + 32 * e + 2 * u + g] = 1.0
    return a12, a3, a4


def kernel(x, t, y, w, b, trace=False):
    N = x.shape[0]
    Npad = M * R
    npad = Npad - N
    assert npad >= 0
    f32 = np.float32

    # features [NF, Npad]: t, x0..x9, one, one (fp8; all exact)
    F = np.zeros((NF, Npad), f32)
    F[0, :N] = np.asarray(t, f32).reshape(-1)
    F[1:11, :N] = np.asarray(x, f32).T
    F[11, :N] = 1.0
    F[12, :N] = 1.0

    xin = np.zeros((M, 128, NPAIR, NT), F8)
    yp = np.concatenate([np.asarray(y, f32).reshape(-1), np.zeros(npad, f32)])
    yin = np.empty((M, NPACK * 16, NT), f32)
    for m in range(M):
        Fm = F[:, m * R:(m + 1) * R].reshape(NF, NPAIR, 2, G, NT)
        # arr[pair, ab, 26, NT]: per-tile feature block (g-major)
        arr = Fm.transpose(1, 2, 3, 0, 4).reshape(NPAIR, 2, 2 * NF, NT).astype(F8)
        for q in range(4):
            xin[m, 32 * q:32 * q + 2 * NF] = arr[:, q // 2].transpose(1, 0, 2)
        yin[m] = yp[m * R:(m + 1) * R].reshape(NPACK * 16, NT)
    xin = xin.reshape(M, 128, NPAIR * NT)

    a12, a3, a4 = _make_consts(w, b)

    nc = _get_program()
    in_maps = [
        {"xin": xin[i], "yin": yin[i], "a12": a12, "a3": a3, "a4": a4}
        for i in range(M)
    ]
    res = run_bass_kernel_spmd(nc, in_maps, core_ids=list(range(M)), trace=trace)
    zfull = np.concatenate(
        [res.results[i]["z"].astype(f32).reshape(-1) for i in range(M)]
    )[:N]
    out = zfull.reshape(N, 1).astype(np.float32)
    if trace:
        return out, res
    return out


# revision 31
# speedup vs baseline: 1.7046x; 1.7046x over previous
"""Trainium2 Bass kernel: per-combination linear encoder via PE one-hot
matmuls, v2 (row-tiled S1 + multi-engine stage 2).

z = y * w[idx] + b[idx],  idx = t*1024 + x @ [512..1]  (11 bits, 2048 combos)

Split idx = hi5*64 + lo6 (hi5 = t,x0..x3; lo6 = x4..x9). Per pair of
tiles (2 x 1024 rows; each tile = 2 groups g of 512 columns, rows on
the FREE axis):

  S1  PE   4 concurrent 32x128 row-tiled fp8 matmuls (tile_position
           quadrants 0/32/64/96) compute uu = [u_lo | u_hi] for both
           tiles of the pair at once, where u_lo[64g+l,c] = lo6-l and
           u_hi[64g+2h+e,c] = hi5-h. Features are replicated in 4 SBUF
           partition quadrants (fp8, so DMA bytes match the old bf16
           single-copy layout). The -l / -h stationary entries are
           split over two ones-features so every value is fp8-exact.
  ACT      drains uu psum -> bf16 SBUF (exact: |u| < 64)
  DVE      oh = (u_lo == 0)            bf16 one-hot (4x mode)
  PE       V[64g+2h+e, c] = sum_l T[l,2h+e]*oh  (T = w/b tables, bf16)
  DVE      msk = (u_hi == 0) * V       fused scalar_tensor_tensor,
                                       u_hi from SBUF, V from psum
  PE       sel8[48,512] accumulates 8 tiles per pack via slot-shifted
           one-hot stationaries (slot 2u+g for w, 32+2u+g for b)
  DVE      z[16,512] = y16 * sel8[0:16] + sel8[32:48]; DMA out.

PE work is software-pipelined: at iteration j the PE runs V(j-1),
S1(j), sel(j-2) so no engine blocks on the serial chain, and the
pack-tail fma is deferred one DVE op per iteration so it never stalls
the critical one-hot/STT chain. PSUM budget: uu 2x2 + V 2 + sel 2 =
8 banks. ACT (the uu drain, ~2.2us/pair) is the pacing engine; DVE is
~2.1us/pair; PE ~2.0us/pair.

All row/column packing is host-side layout only; the device does all
arithmetic. w/b are bf16 (0.2% rounding, tolerance is 2e-2).
"""

import numpy as np
import ml_dtypes

import concourse.bacc as bacc
import concourse.mybir as mybir
from concourse.tile import TileContext
from concourse.bass_utils import run_bass_kernel_spmd

M = 8              # NeuronCores
NT = 512           # columns per tile (one PSUM bank)
G = 2              # row-groups per column
TPP = 8            # tiles per pack (sel8 accumulation group)
NPACK = 31         # packs per core
NTILES = NPACK * TPP          # 248
NPAIR = NTILES // 2           # 124
RPT = G * NT                  # rows per tile (1024)
R = NTILES * RPT              # rows per core (253952)
C = 2048
NF = 13            # features per group: t, x0..x9, one, one
F32 = mybir.dt.float32
BF16 = mybir.dt.bfloat16
FP8 = mybir.dt.float8e4
BF = ml_dtypes.bfloat16
F8 = ml_dtypes.float8_e4m3

OH_ON_GPSIMD = False

_CACHE = {}


def _build_program():
    nc = bacc.Bacc("TRN2", target_bir_lowering=False, debug=False, num_devices=M)

    xin = nc.dram_tensor("xin", [128, NPAIR * NT], FP8, kind="ExternalInput")
    yin = nc.dram_tensor("yin", [NPACK * 16, NT], F32, kind="ExternalInput")
    a12 = nc.dram_tensor("a12", [128, 128], FP8, kind="ExternalInput")
    a3 = nc.dram_tensor("a3", [128, 128], BF16, kind="ExternalInput")
    a4 = nc.dram_tensor("a4", [128, TPP * 48], BF16, kind="ExternalInput")
    z = nc.dram_tensor("z", [NPACK * 16, NT], F32, kind="ExternalOutput")

    isq = mybir.AluOpType.is_equal
    mul = mybir.AluOpType.mult
    add = mybir.AluOpType.add

    with TileContext(nc) as tc:
        with (
            tc.tile_pool(name="const", bufs=1) as cpool,
            tc.tile_pool(name="inx", bufs=3) as ipool,
            tc.tile_pool(name="iny", bufs=3) as ypool,
            tc.tile_pool(name="mid", bufs=6) as spool,
            tc.tile_pool(name="ohp", bufs=6) as ohpool,
            tc.tile_pool(name="msk", bufs=8) as mpool,
            tc.tile_pool(name="out", bufs=2) as opool,
            tc.tile_pool(name="psu", bufs=2, space="PSUM") as ppu,
            tc.tile_pool(name="psv", bufs=2, space="PSUM") as ppv,
            tc.tile_pool(name="pss", bufs=2, space="PSUM") as pps,
        ):
            a12_t = cpool.tile([128, 128], FP8)
            nc.sync.dma_start(out=a12_t[:], in_=a12[:, :])
            a3_t = cpool.tile([128, 128], BF16)
            nc.sync.dma_start(out=a3_t[:], in_=a3[:, :])
            a4_t = cpool.tile([128, TPP * 48], BF16)
            nc.sync.dma_start(out=a4_t[:], in_=a4[:, :])

            xts = {}
            yts = {}

            def fetch_pack(k):
                xt = ipool.tile([128, (TPP // 2) * NT], FP8, tag="x", name="xt")
                base = k * (TPP // 2) * NT
                for s in range(4):
                    nc.sync.dma_start(
                        out=xt[:, s * NT:(s + 1) * NT],
                        in_=xin[:, base + s * NT:base + (s + 1) * NT],
                    )
                yt = ypool.tile([16, NT], F32, tag="y", name="yt")
                nc.sync.dma_start(out=yt[:], in_=yin[16 * k:16 * (k + 1), :])
                xts[k] = xt
                yts[k] = yt

            sel8_box = [None]
            tail_q = []

            def issue_tail(s8, yt_, pk_):
                # Deferred: one DVE op per later iteration so the pack
                # tail never stalls the critical one-hot/STT chain.
                zbox = [None]

                def c1():
                    zbox[0] = opool.tile([16, NT], F32, tag="z", name="zt")
                    nc.vector.tensor_tensor(
                        out=zbox[0][:], in0=yt_[:], in1=s8[0:16, :], op=mul
                    )

                def c2():
                    nc.vector.tensor_tensor(
                        out=zbox[0][:], in0=zbox[0][:], in1=s8[32:48, :], op=add
                    )
                    nc.sync.dma_start(
                        out=z[16 * pk_:16 * (pk_ + 1), :], in_=zbox[0][:]
                    )

                tail_q.append(c1)
                tail_q.append(c2)

            def issue_V(st):
                oh_l, uub_l, u0, yt_, pk_ = st
                V_l = [ppv.tile([128, NT], F32, tag="V", name="V") for _ in range(2)]
                msk_l = [
                    mpool.tile([128, NT], BF16, tag="msk", name="msk") for _ in range(2)
                ]
                for i in range(2):
                    nc.tensor.matmul(
                        out=V_l[i][:], lhsT=a3_t[:], rhs=oh_l[i][:],
                        start=True, stop=True,
                    )
                for i in range(2):
                    nc.vector.scalar_tensor_tensor(
                        out=msk_l[i][:], in0=uub_l[i][:, NT:2 * NT], scalar=0.0,
                        in1=V_l[i][:], op0=isq, op1=mul,
                    )
                return (msk_l, u0, yt_, pk_)

            def issue_sel(st):
                msk_l, u0, yt_, pk_ = st
                for i in range(2):
                    u = (u0 + i) % TPP
                    if u == 0:
                        sel8_box[0] = pps.tile(
                            [64, NT], F32, tag="sel", name="sel8"
                        )
                    nc.tensor.matmul(
                        out=sel8_box[0][0:48, :],
                        lhsT=a4_t[:, 48 * u:48 * (u + 1)],
                        rhs=msk_l[i][:],
                        start=(u == 0), stop=(u == TPP - 1),
                    )
                if (u0 + 1) % TPP == TPP - 1:
                    issue_tail(sel8_box[0], yt_, pk_)

            pend_v = None
            pend_sel = None

            fetch_pack(0)
            for pk in range(NPACK):
                if pk + 1 < NPACK:
                    fetch_pack(pk + 1)
                xt = xts.pop(pk)
                yt = yts[pk]

                for p in range(TPP // 2):
                    u0 = pk * TPP + 2 * p

                    new_sel = issue_V(pend_v) if pend_v is not None else None

                    # S1: 4 concurrent row-tiled matmuls for this pair.
                    # Tile i's uu = [u_lo | u_hi] (two psum banks).
                    xq = xt[:, p * NT:(p + 1) * NT]
                    uu_l = []
                    for i in range(2):
                        qb = 64 * i
                        uu = ppu.tile([128, 2 * NT], F32, tag="uu", name="uu")
                        nc.tensor.matmul(
                            out=uu[:, 0:NT], lhsT=a12_t[qb:qb + 2 * NF, :],
                            rhs=xq[qb:qb + 2 * NF, :],
                            start=True, stop=True, tile_position=(qb, 0),
                        )
                        nc.tensor.matmul(
                            out=uu[:, NT:2 * NT],
                            lhsT=a12_t[qb + 32:qb + 32 + 2 * NF, :],
                            rhs=xq[qb + 32:qb + 32 + 2 * NF, :],
                            start=True, stop=True, tile_position=(qb + 32, 0),
                        )
                        uu_l.append(uu)

                    if pend_sel is not None:
                        issue_sel(pend_sel)
                    pend_sel = new_sel

                    # ACT drain of uu, then one-hot compare on u_lo
                    oh_l, uub_l = [], []
                    for i in range(2):
                        uub = spool.tile([128, 2 * NT], BF16, tag="uub", name="uub")
                        nc.scalar.copy(out=uub[:], in_=uu_l[i][:])
                        oh = ohpool.tile([128, NT], BF16, tag="oh", name="oh")
                        eng = nc.gpsimd if OH_ON_GPSIMD else nc.vector
                        eng.tensor_scalar(
                            out=oh[:], in0=uub[:, 0:NT], scalar1=0.0,
                            scalar2=None, op0=isq,
                        )
                        oh_l.append(oh)
                        uub_l.append(uub)

                    if len(tail_q) > 1 or (tail_q and u0 % TPP == 4):
                        tail_q.pop(0)()

                    pend_v = (oh_l, uub_l, u0, yt, pk)

            # drain the software pipeline
            new_sel = issue_V(pend_v)
            issue_sel(pend_sel)
            issue_sel(new_sel)
            while tail_q:
                tail_q.pop(0)()

    nc.compile()
    return nc


def _get_program():
    if "nc" not in _CACHE:
        _CACHE["nc"] = _build_program()
    return _CACHE["nc"]


def _make_consts(w, b):
    f32 = np.float32
    wb_ = np.stack([np.asarray(w, f32), np.asarray(b, f32)], 1).astype(BF)  # [2048, 2]

    # a12: 4 quadrants of fp8 stationaries. Quadrants 0/2: a1 (lo6),
    # quadrants 1/3: a2 (hi5). All entries fp8-exact.
    a12 = np.zeros((128, 128), f32)
    lvals = np.arange(64, dtype=f32)
    hvals = np.repeat(np.arange(32, dtype=f32), 2)
    for q in range(4):
        qb = 32 * q
        for g in range(G):
            cb = 64 * g
            if q % 2 == 0:  # a1: lo6 from x4..x9 (feature rows 5..10)
                for si in range(6):
                    a12[qb + NF * g + 5 + si, cb:cb + 64] = 2.0 ** (5 - si)
                a12[qb + NF * g + 11, cb:cb + 64] = -(lvals - (lvals % 4))
                a12[qb + NF * g + 12, cb:cb + 64] = -(lvals % 4)
            else:  # a2: hi5 from t,x0..x3 (feature rows 0..4)
                for si in range(5):
                    a12[qb + NF * g + si, cb:cb + 64] = 2.0 ** (4 - si)
                a12[qb + NF * g + 11, cb:cb + 64] = -(hvals - (hvals % 4))
                a12[qb + NF * g + 12, cb:cb + 64] = -(hvals % 4)
    a12 = a12.astype(F8)

    a3 = np.zeros((128, 128), BF)
    for g in range(G):
        for h in range(32):
            for e in range(2):
                a3[64 * g:64 * g + 64, 64 * g + 2 * h + e] = wb_[h * 64:(h + 1) * 64, e]
    # sel8 slot for tile u, group g: w at partition 2u+g, b at 32+2u+g
    a4 = np.zeros((128, TPP * 48), BF)
    for u in range(TPP):
        for g in range(G):
            for e in range(2):
                for h in range(32):
                    a4[64 * g + 2 * h + e, 48 * u + 32 * e + 2 * u + g] = 1.0
    return a12, a3, a4


def kernel(x, t, y, w, b, trace=False):
    N = x.shape[0]
    Npad = M * R
    npad = Npad - N
    assert npad >= 0
    f32 = np.float32

    # features [NF, Npad]: t, x0..x9, one, one (fp8; all exact)
    F = np.zeros((NF, Npad), f32)
    F[0, :N] = np.asarray(t, f32).reshape(-1)
    F[1:11, :N] = np.asarray(x, f32).T
    F[11, :N] = 1.0
    F[12, :N] = 1.0

    xin = np.zeros((M, 128, NPAIR, NT), F8)
    yp = np.concatenate([np.asarray(y, f32).reshape(-1), np.zeros(npad, f32)])
    yin = np.empty((M, NPACK * 16, NT), f32)
    for m in range(M):
        Fm = F[:, m * R:(m + 1) * R].reshape(NF, NPAIR, 2, G, NT)
        # arr[pair, ab, 26, NT]: per-tile feature block (g-major)
        arr = Fm.transpose(1, 2, 3, 0, 4).reshape(NPAIR, 2, 2 * NF, NT).astype(F8)
        for q in range(4):
            xin[m, 32 * q:32 * q + 2 * NF] = arr[:, q // 2].transpose(1, 0, 2)
        yin[m] = yp[m * R:(m + 1) * R].reshape(NPACK * 16, NT)
    xin = xin.reshape(M, 128, NPAIR * NT)

    a12, a3, a4 = _make_consts(w, b)

    nc = _get_program()
    in_maps = [
        {"xin": xin[i], "yin": yin[i], "a12": a12, "a3": a3, "a4": a4}
        for i in range(M)
    ]
    res = run_bass_kernel_spmd(nc, in_maps, core_ids=list(range(M)), trace=trace)
    zfull = np.concatenate(
        [res.results[i]["z"].astype(f32).reshape(-1) for i in range(M)]
    )[:N]
    out = zfull.reshape(N, 1).astype(np.float32)
    if trace:
        return out, res
    return out


# revision 32
# speedup vs baseline: 1.7082x; 1.0021x over previous
"""Trainium2 Bass kernel: per-combination linear encoder via PE one-hot
matmuls, v2 (row-tiled S1 + multi-engine stage 2).

z = y * w[idx] + b[idx],  idx = t*1024 + x @ [512..1]  (11 bits, 2048 combos)

Split idx = hi5*64 + lo6 (hi5 = t,x0..x3; lo6 = x4..x9). Per pair of
tiles (2 x 1024 rows; each tile = 2 groups g of 512 columns, rows on
the FREE axis):

  S1  PE   4 concurrent 32x128 row-tiled fp8 matmuls (tile_position
           quadrants 0/32/64/96) compute uu = [u_lo | u_hi] for both
           tiles of the pair at once, where u_lo[64g+l,c] = lo6-l and
           u_hi[64g+2h+e,c] = hi5-h. Features are replicated in 4 SBUF
           partition quadrants (fp8, so DMA bytes match the old bf16
           single-copy layout). The -l / -h stationary entries are
           split over two ones-features so every value is fp8-exact.
  ACT      drains uu psum -> bf16 SBUF (exact: |u| < 64)
  DVE      oh = (u_lo == 0)            bf16 one-hot (4x mode)
  PE       V[64g+2h+e, c] = sum_l T[l,2h+e]*oh  (T = w/b tables, bf16)
  DVE      msk = (u_hi == 0) * V       fused scalar_tensor_tensor,
                                       u_hi from SBUF, V from psum
  PE       sel8[48,512] accumulates 8 tiles per pack via slot-shifted
           one-hot stationaries (slot 2u+g for w, 32+2u+g for b)
  DVE      z[16,512] = y16 * sel8[0:16] + sel8[32:48]; DMA out.

PE work is software-pipelined: at iteration j the PE runs V(j-1),
S1(j), sel(j-2) so no engine blocks on the serial chain, and the
pack-tail fma is deferred one DVE op per iteration so it never stalls
the critical one-hot/STT chain. PSUM budget: uu 2x2 + V 2 + sel 2 =
8 banks. ACT (the uu drain, ~2.2us/pair) is the pacing engine; DVE is
~2.1us/pair; PE ~2.0us/pair.

All row/column packing is host-side layout only; the device does all
arithmetic. w/b are bf16 (0.2% rounding, tolerance is 2e-2).
"""

import numpy as np
import ml_dtypes

import concourse.bacc as bacc
import concourse.mybir as mybir
from concourse.tile import TileContext
from concourse.bass_utils import run_bass_kernel_spmd

M = 8              # NeuronCores
NT = 512           # columns per tile (one PSUM bank)
G = 2              # row-groups per column
TPP = 8            # tiles per pack (sel8 accumulation group)
NPACK = 31         # packs per core
NTILES = NPACK * TPP          # 248
NPAIR = NTILES // 2           # 124
RPT = G * NT                  # rows per tile (1024)
R = NTILES * RPT              # rows per core (253952)
C = 2048
NF = 13            # features per group: t, x0..x9, one, one
F32 = mybir.dt.float32
BF16 = mybir.dt.bfloat16
FP8 = mybir.dt.float8e4
BF = ml_dtypes.bfloat16
F8 = ml_dtypes.float8_e4m3

OH_ON_GPSIMD = False

_CACHE = {}


def _build_program():
    nc = bacc.Bacc("TRN2", target_bir_lowering=False, debug=False, num_devices=M)

    xin = nc.dram_tensor("xin", [128, NPAIR * NT], FP8, kind="ExternalInput")
    yin = nc.dram_tensor("yin", [NPACK * 16, NT], F32, kind="ExternalInput")
    a12 = nc.dram_tensor("a12", [128, 128], FP8, kind="ExternalInput")
    a3 = nc.dram_tensor("a3", [128, 128], BF16, kind="ExternalInput")
    a4 = nc.dram_tensor("a4", [128, TPP * 48], BF16, kind="ExternalInput")
    z = nc.dram_tensor("z", [NPACK * 16, NT], F32, kind="ExternalOutput")

    isq = mybir.AluOpType.is_equal
    mul = mybir.AluOpType.mult
    add = mybir.AluOpType.add

    with TileContext(nc) as tc:
        with (
            tc.tile_pool(name="const", bufs=1) as cpool,
            tc.tile_pool(name="inx", bufs=4) as ipool,
            tc.tile_pool(name="iny", bufs=4) as ypool,
            tc.tile_pool(name="mid", bufs=8) as spool,
            tc.tile_pool(name="ohp", bufs=8) as ohpool,
            tc.tile_pool(name="msk", bufs=10) as mpool,
            tc.tile_pool(name="out", bufs=3) as opool,
            tc.tile_pool(name="psu", bufs=2, space="PSUM") as ppu,
            tc.tile_pool(name="psv", bufs=2, space="PSUM") as ppv,
            tc.tile_pool(name="pss", bufs=2, space="PSUM") as pps,
        ):
            a12_t = cpool.tile([128, 128], FP8)
            nc.sync.dma_start(out=a12_t[:], in_=a12[:, :])
            a3_t = cpool.tile([128, 128], BF16)
            nc.sync.dma_start(out=a3_t[:], in_=a3[:, :])
            a4_t = cpool.tile([128, TPP * 48], BF16)
            nc.sync.dma_start(out=a4_t[:], in_=a4[:, :])

            xts = {}
            yts = {}

            def fetch_pack(k):
                xt = ipool.tile([128, (TPP // 2) * NT], FP8, tag="x", name="xt")
                base = k * (TPP // 2) * NT
                for s in range(4):
                    nc.sync.dma_start(
                        out=xt[:, s * NT:(s + 1) * NT],
                        in_=xin[:, base + s * NT:base + (s + 1) * NT],
                    )
                yt = ypool.tile([16, NT], F32, tag="y", name="yt")
                nc.sync.dma_start(out=yt[:], in_=yin[16 * k:16 * (k + 1), :])
                xts[k] = xt
                yts[k] = yt

            sel8_box = [None]
            tail_q = []

            def issue_tail(s8, yt_, pk_):
                # Deferred: one DVE op per later iteration so the pack
                # tail never stalls the critical one-hot/STT chain.
                zbox = [None]

                def c1():
                    zbox[0] = opool.tile([16, NT], F32, tag="z", name="zt")
                    nc.vector.tensor_tensor(
                        out=zbox[0][:], in0=yt_[:], in1=s8[0:16, :], op=mul
                    )

                def c2():
                    nc.vector.tensor_tensor(
                        out=zbox[0][:], in0=zbox[0][:], in1=s8[32:48, :], op=add
                    )
                    nc.sync.dma_start(
                        out=z[16 * pk_:16 * (pk_ + 1), :], in_=zbox[0][:]
                    )

                tail_q.append(c1)
                tail_q.append(c2)

            def issue_V(st):
                oh_l, uub_l, u0, yt_, pk_ = st
                V_l = [ppv.tile([128, NT], F32, tag="V", name="V") for _ in range(2)]
                msk_l = [
                    mpool.tile([128, NT], BF16, tag="msk", name="msk") for _ in range(2)
                ]
                for i in range(2):
                    nc.tensor.matmul(
                        out=V_l[i][:], lhsT=a3_t[:], rhs=oh_l[i][:],
                        start=True, stop=True,
                    )
                for i in range(2):
                    nc.vector.scalar_tensor_tensor(
                        out=msk_l[i][:], in0=uub_l[i][:, NT:2 * NT], scalar=0.0,
                        in1=V_l[i][:], op0=isq, op1=mul,
                    )
                return (msk_l, u0, yt_, pk_)

            def issue_sel(st):
                msk_l, u0, yt_, pk_ = st
                for i in range(2):
                    u = (u0 + i) % TPP
                    if u == 0:
                        sel8_box[0] = pps.tile(
                            [64, NT], F32, tag="sel", name="sel8"
                        )
                    nc.tensor.matmul(
                        out=sel8_box[0][0:48, :],
                        lhsT=a4_t[:, 48 * u:48 * (u + 1)],
                        rhs=msk_l[i][:],
                        start=(u == 0), stop=(u == TPP - 1),
                    )
                if (u0 + 1) % TPP == TPP - 1:
                    issue_tail(sel8_box[0], yt_, pk_)

            pend_v = None
            pend_sel = None

            fetch_pack(0)
            for pk in range(NPACK):
                if pk + 1 < NPACK:
                    fetch_pack(pk + 1)
                xt = xts.pop(pk)
                yt = yts[pk]

                for p in range(TPP // 2):
                    u0 = pk * TPP + 2 * p

                    new_sel = issue_V(pend_v) if pend_v is not None else None

                    # S1: 4 concurrent row-tiled matmuls for this pair.
                    # Tile i's uu = [u_lo | u_hi] (two psum banks).
                    xq = xt[:, p * NT:(p + 1) * NT]
                    uu_l = []
                    for i in range(2):
                        qb = 64 * i
                        uu = ppu.tile([128, 2 * NT], F32, tag="uu", name="uu")
                        nc.tensor.matmul(
                            out=uu[:, 0:NT], lhsT=a12_t[qb:qb + 2 * NF, :],
                            rhs=xq[qb:qb + 2 * NF, :],
                            start=True, stop=True, tile_position=(qb, 0),
                        )
                        nc.tensor.matmul(
                            out=uu[:, NT:2 * NT],
                            lhsT=a12_t[qb + 32:qb + 32 + 2 * NF, :],
                            rhs=xq[qb + 32:qb + 32 + 2 * NF, :],
                            start=True, stop=True, tile_position=(qb + 32, 0),
                        )
                        uu_l.append(uu)

                    if pend_sel is not None:
                        issue_sel(pend_sel)
                    pend_sel = new_sel

                    # ACT drain of uu, then one-hot compare on u_lo
                    oh_l, uub_l = [], []
                    for i in range(2):
                        uub = spool.tile([128, 2 * NT], BF16, tag="uub", name="uub")
                        nc.scalar.copy(out=uub[:], in_=uu_l[i][:])
                        oh = ohpool.tile([128, NT], BF16, tag="oh", name="oh")
                        eng = nc.gpsimd if OH_ON_GPSIMD else nc.vector
                        eng.tensor_scalar(
                            out=oh[:], in0=uub[:, 0:NT], scalar1=0.0,
                            scalar2=None, op0=isq,
                        )
                        oh_l.append(oh)
                        uub_l.append(uub)

                    if len(tail_q) > 1 or (tail_q and u0 % TPP == 4):
                        tail_q.pop(0)()

                    pend_v = (oh_l, uub_l, u0, yt, pk)

            # drain the software pipeline
            new_sel = issue_V(pend_v)
            issue_sel(pend_sel)
            issue_sel(new_sel)
            while tail_q:
                tail_q.pop(0)()

    nc.compile()
    return nc


def _get_program():
    if "nc" not in _CACHE:
        _CACHE["nc"] = _build_program()
    return _CACHE["nc"]


def _make_consts(w, b):
    f32 = np.float32
    wb_ = np.stack([np.asarray(w, f32), np.asarray(b, f32)], 1).astype(BF)  # [2048, 2]

    # a12: 4 quadrants of fp8 stationaries. Quadrants 0/2: a1 (lo6),
    # quadrants 1/3: a2 (hi5). All entries fp8-exact.
    a12 = np.zeros((128, 128), f32)
    lvals = np.arange(64, dtype=f32)
    hvals = np.repeat(np.arange(32, dtype=f32), 2)
    for q in range(4):
        qb = 32 * q
        for g in range(G):
            cb = 64 * g
            if q % 2 == 0:  # a1: lo6 from x4..x9 (feature rows 5..10)
                for si in range(6):
                    a12[qb + NF * g + 5 + si, cb:cb + 64] = 2.0 ** (5 - si)
                a12[qb + NF * g + 11, cb:cb + 64] = -(lvals - (lvals % 4))
                a12[qb + NF * g + 12, cb:cb + 64] = -(lvals % 4)
            else:  # a2: hi5 from t,x0..x3 (feature rows 0..4)
                for si in range(5):
                    a12[qb + NF * g + si, cb:cb + 64] = 2.0 ** (4 - si)
                a12[qb + NF * g + 11, cb:cb + 64] = -(hvals - (hvals % 4))
                a12[qb + NF * g + 12, cb:cb + 64] = -(hvals % 4)
    a12 = a12.astype(F8)

    a3 = np.zeros((128, 128), BF)
    for g in range(G):
        for h in range(32):
            for e in range(2):
                a3[64 * g:64 * g + 64, 64 * g + 2 * h + e] = wb_[h * 64:(h + 1) * 64, e]
    # sel8 slot for tile u, group g: w at partition 2u+g, b at 32+2u+g
    a4 = np.zeros((128, TPP * 48), BF)
    for u in range(TPP):
        for g in range(G):
            for e in range(2):
                for h in range(32):
                    a4[64 * g + 2 * h + e, 48 * u + 32 * e + 2 * u + g] = 1.0
    return a12, a3, a4


def kernel(x, t, y, w, b, trace=False):
    N = x.shape[0]
    Npad = M * R
    npad = Npad - N
    assert npad >= 0
    f32 = np.float32

    # features [NF, Npad]: t, x0..x9, one, one (fp8; all exact)
    F = np.zeros((NF, Npad), f32)
    F[0, :N] = np.asarray(t, f32).reshape(-1)
    F[1:11, :N] = np.asarray(x, f32).T
    F[11, :N] = 1.0
    F[12, :N] = 1.0

    xin = np.zeros((M, 128, NPAIR, NT), F8)
    yp = np.concatenate([np.asarray(y, f32).reshape(-1), np.zeros(npad, f32)])
    yin = np.empty((M, NPACK * 16, NT), f32)
    for m in range(M):
        Fm = F[:, m * R:(m + 1) * R].reshape(NF, NPAIR, 2, G, NT)
        # arr[pair, ab, 26, NT]: per-tile feature block (g-major)
        arr = Fm.transpose(1, 2, 3, 0, 4).reshape(NPAIR, 2, 2 * NF, NT).astype(F8)
        for q in range(4):
            xin[m, 32 * q:32 * q + 2 * NF] = arr[:, q // 2].transpose(1, 0, 2)
        yin[m] = yp[m * R:(m + 1) * R].reshape(NPACK * 16, NT)
    xin = xin.reshape(M, 128, NPAIR * NT)

    a12, a3, a4 = _make_consts(w, b)

    nc = _get_program()
    in_maps = [
        {"xin": xin[i], "yin": yin[i], "a12": a12, "a3": a3, "a4": a4}
        for i in range(M)
    ]
    res = run_bass_kernel_spmd(nc, in_maps, core_ids=list(range(M)), trace=trace)
    zfull = np.concatenate(
        [res.results[i]["z"].astype(f32).reshape(-1) for i in range(M)]
    )[:N]
    out = zfull.reshape(N, 1).astype(np.float32)
    if trace:
        return out, res
    return out
